# revision 29
# baseline (speedup 1.0000x reference)
"""Trainium2 Bass kernel for GQA sliding-window attention (nn_Attention_20375324852422).

Reference computation (B=2, T=2048, D=2560, N=8 q-heads, K=4 kv-heads, H=256,
WINDOW=1024):
    q = x @ q_w ; k,v = x @ kv_w      (GQA projections)
    q,k = rms_norm(q/k, scale)        (QK-norm, (1+scale) gain)
    q,k = rope(q/k, positions); q *= H**-0.5
    logits = q @ k.T  (grouped, sliding-window causal mask), softmax
    out = (probs @ v) @ out_w

Sharding: 8 cores = 2 (batch) x 4 (kv-heads).  Each core owns one batch row and
one kv head (plus its two grouped q heads) over the FULL sequence, so no
projection work is replicated anywhere (the old seq-chunk layout recomputed K/V
3x).  Each core emits a partial output (its 2 heads' contribution through
out_w); the host sums the 4 partials per batch row - no collectives.

Per-core device layouts (host prepares, see kernel()):
    xT   [D, 2048]   x[b] transposed (bf16)       - lhsT for projections
    kvw  [D, 512]    [kw | vw] for the kv head    - fused rhs (one N=512 matmul
    qw   [D, 512]    2 q heads                      chain per seq tile)
    ow   [512, D]    out_w rows for the 2 heads
    tq   [2048, 4, 128] / tk [...]                - RoPE tables (C1,S1,C2,S2)
                     with (1+scale) gains and (for q) H**-0.5 folded in
    tri  [2, 128, 128] {0,1} triangle masks: [0]=diag tile (key<=query),
                     [1]=far tile (key>query)  -- the sliding-window mask is
                     canonical (host-checked; numpy fallback otherwise)
    out  [2048, D]   fp32 partial
"""

import numpy as np
import ml_dtypes

import concourse.bass as bass
import concourse.tile as tile
from concourse import bacc
from concourse import mybir
from concourse.masks import make_identity

BF16 = mybir.dt.bfloat16
F32 = mybir.dt.float32

B, T, D, N, K, H = 2, 2048, 2560, 8, 4, 256
G = N // K
WINDOW = 1024
ROPE_BASE = 10000
EPS = 1e-6
HH = H // 2  # 128

D_TILES = D // 128   # 20
S_TILES = T // 128   # 16
NCH = T // 512       # 4 query chunks of 512
WT = WINDOW // 128   # 8


def build_nc():
    """Per-core Bass graph: full-T GQA attention for 2 q-heads / 1 kv-head."""
    nc = bacc.Bacc()
    nh = 2

    xT_e = nc.dram_tensor("xT", [D, T], BF16, kind="ExternalInput")
    kvw_e = nc.dram_tensor("kvw", [D, 2 * H], BF16, kind="ExternalInput")
    qw_e = nc.dram_tensor("qw", [D, nh * H], BF16, kind="ExternalInput")
    ow_e = nc.dram_tensor("ow", [nh * H, D], BF16, kind="ExternalInput")
    tq_e = nc.dram_tensor("tq", [T, 4, HH], BF16, kind="ExternalInput")
    tk_e = nc.dram_tensor("tk", [T, 4, HH], BF16, kind="ExternalInput")
    tri_e = nc.dram_tensor("tri", [2, 128, 128], BF16, kind="ExternalInput")
    out_e = nc.dram_tensor("out", [T, D], F32, kind="ExternalOutput")

    with tile.TileContext(nc) as tc:
        with (
            tc.tile_pool(name="const", bufs=1) as const,
            tc.tile_pool(name="persist", bufs=1) as persist,
            tc.tile_pool(name="psA", bufs=1, space="PSUM") as psA,
            tc.tile_pool(name="psT", bufs=2, space="PSUM") as psT,
        ):
            ident = const.tile([128, 128], BF16)
            make_identity(nc, ident)
            eps_t = const.tile([128, 1], F32)
            nc.vector.memset(eps_t, EPS)
            tri_sb = const.tile([128, 2, 128], BF16)

            kT = persist.tile([128, 2, T], BF16)          # [h, kv half, s]
            v_sb = persist.tile([128, S_TILES, H + 1], BF16)
            qT = persist.tile([128, nh * 2, T], BF16)     # [h, head*2+half, s]


            def rope(dst, src, tbl, heads):
                """dst/src: [128, heads, H] sbuf; tbl: [128, 4, HH] slice."""
                first = src[:, :, 0:HH]
                second = src[:, :, HH:H]
                c1 = tbl[:, 0, :].unsqueeze(1).broadcast_to([128, heads, HH])
                s1 = tbl[:, 1, :].unsqueeze(1).broadcast_to([128, heads, HH])
                c2 = tbl[:, 2, :].unsqueeze(1).broadcast_to([128, heads, HH])
                s2 = tbl[:, 3, :].unsqueeze(1).broadcast_to([128, heads, HH])
                t1 = scratch.tile([128, heads, HH], BF16, tag="rp1", name="t1")
                t2 = scratch.tile([128, heads, HH], BF16, tag="rp2", name="t2")
                nc.vector.tensor_mul(t1, first, c1)
                nc.vector.tensor_mul(t2, second, s1)
                nc.vector.tensor_sub(dst[:, :, 0:HH], t1, t2)
                nc.vector.tensor_mul(t1, second, c2)
                nc.vector.tensor_mul(t2, first, s2)
                nc.vector.tensor_add(dst[:, :, HH:H], t1, t2)

            def norm_scale_copy(dst, psrc, heads):
                """RMS-normalize psum [128, heads*H] into sbuf dst [128, heads, H]."""
                ssq = scratch.tile([128, heads], F32, tag="ssq", name="ssq")
                sq_junk = scratch.tile([128, H], BF16, tag="sqj", name="sqj")
                for hh in range(heads):
                    nc.scalar.activation(
                        out=sq_junk, in_=psrc[:, hh * H:(hh + 1) * H],
                        func=mybir.ActivationFunctionType.Square,
                        accum_out=ssq[:, hh:hh + 1])
                nc.scalar.activation(
                    out=ssq, in_=ssq, func=mybir.ActivationFunctionType.Sqrt,
                    bias=eps_t, scale=1.0 / H)
                nc.vector.reciprocal(ssq, ssq)
                for hh in range(heads):
                    nc.scalar.activation(
                        out=dst[:, hh, :], in_=psrc[:, hh * H:(hh + 1) * H],
                        func=mybir.ActivationFunctionType.Copy,
                        scale=ssq[:, hh:hh + 1])

            tk_sb = persist.tile([128, S_TILES, 4, HH], BF16, name="tk_sb")
            tq_sb = persist.tile([128, S_TILES, 4, HH], BF16, name="tq_sb")
            DEF = 8   # rope/transpose deferral depth (tolerates ~70us of
                      # table-DMA latency without stalling any engine stream)

            scratch_cm = tc.tile_pool(name="scratch", bufs=2)
            scratch = scratch_cm.__enter__()
            q_ns = {}
            k_ns = {}

            def k_finish(st):
                k_n = k_ns.pop(st)
                k_r = scratch.tile([128, 1, H], BF16, tag="x_r", bufs=2,
                                   name="k_r")
                rope(k_r, k_n, tk_sb[:, st], 1)
                pt = psT.tile([128, 2, 128], BF16, tag="pt", name="pt")
                for half in range(2):
                    nc.tensor.transpose(
                        pt[:, half], k_r[:, 0, half * HH:(half + 1) * HH],
                        ident)
                nc.vector.tensor_copy(kT[:, :, st * 128:(st + 1) * 128], pt)

            def q_finish(st):
                q_n = q_ns.pop(st)
                q_r = scratch.tile([128, nh, H], BF16, tag="q_r", bufs=2,
                                   name="q_r")
                rope(q_r, q_n, tq_sb[:, st], nh)
                pt = psT.tile([128, nh * 2, 128], BF16, tag="pt", bufs=2,
                              name="ptq")
                for hh in range(nh):
                    for half in range(2):
                        nc.tensor.transpose(
                            pt[:, hh * 2 + half],
                            q_r[:, hh, half * HH:(half + 1) * HH], ident)
                nc.vector.tensor_copy(qT[:, :, st * 128:(st + 1) * 128], pt)

            with (
                tc.tile_pool(name="xpool", bufs=1) as xpool,
                tc.tile_pool(name="wpool", bufs=1) as wpool,
            ):
                xT_sb = xpool.tile([128, D_TILES, T], BF16, name="xT_sb")

                # Act queue: ONLY the weights (quarter-interleaved so both
                # fused chains start progressively), then tables, then mask.
                # Its instruction stream blocks on dma-queue capacity, so it
                # must carry few transfers and nothing bulky early.
                kvw_sb = wpool.tile([128, D_TILES, 2 * H], BF16, name="kvw_sb")
                qw_sb = wpool.tile([128, D_TILES, nh * H], BF16, name="qw_sb")
                for g in range(4):
                    sl = slice(g * 5, (g + 1) * 5)
                    dsl = slice(g * 640, (g + 1) * 640)
                    nc.scalar.dma_start(
                        out=kvw_sb[:, sl],
                        in_=kvw_e[dsl].rearrange("(t p) c -> p t c", p=128))
                    nc.scalar.dma_start(
                        out=qw_sb[:, sl],
                        in_=qw_e[dsl].rearrange("(t p) c -> p t c", p=128))
                nc.scalar.dma_start(
                    out=tk_sb, in_=tk_e.rearrange("(t p) f h -> p t f h", p=128))
                nc.scalar.dma_start(
                    out=tq_sb, in_=tq_e.rearrange("(t p) f h -> p t f h", p=128))
                nc.scalar.dma_start(out=tri_sb, in_=tri_e.rearrange("t p q -> p t q"))
                # tail x columns ride the Act queue after the tables (chains
                # only reach them at ~115us)
                for lo in (1536, 1792):
                    nc.scalar.dma_start(
                        out=xT_sb[:, :, lo:lo + 256],
                        in_=xT_e[:, lo:lo + 256].rearrange(
                            "(t p) c -> p t c", p=128))

                # bulk xT on the SP queue, strictly column-progressive (each
                # fused chain st consumes one 128-col slice of ALL d-tiles)
                for dt in range(D_TILES):
                    nc.sync.dma_start(
                        out=xT_sb[:, dt, 0:256],
                        in_=xT_e[dt * 128:(dt + 1) * 128, 0:256])
                for lo in range(256, 1536, 256):
                    nc.sync.dma_start(
                        out=xT_sb[:, :, lo:lo + 256],
                        in_=xT_e[:, lo:lo + 256].rearrange(
                            "(t p) c -> p t c", p=128))

                # ---- fused per-st K/V + Q projections ----
                for st in range(S_TILES):
                    pkv = psA.tile([128, 2 * H], F32, tag="pa0", bufs=4, name="pkv")
                    for dt in range(D_TILES):
                        nc.tensor.matmul(pkv, xT_sb[:, dt, st * 128:(st + 1) * 128],
                                         kvw_sb[:, dt, :],
                                         start=(dt == 0), stop=(dt == D_TILES - 1))
                    pq = psA.tile([128, nh * H], F32, tag="pa0", bufs=4, name="pq")
                    for dt in range(D_TILES):
                        nc.tensor.matmul(pq, xT_sb[:, dt, st * 128:(st + 1) * 128],
                                         qw_sb[:, dt, :],
                                         start=(dt == 0), stop=(dt == D_TILES - 1))
                    if st >= DEF:
                        k_finish(st - DEF)
                        q_finish(st - DEF)
                    k_n = scratch.tile([128, 1, H], BF16, tag="k_n",
                                       bufs=DEF + 1, name="k_n")
                    norm_scale_copy(k_n, pkv[:, 0:H], 1)
                    k_ns[st] = k_n
                    nc.vector.tensor_copy(v_sb[:, st, 0:H], pkv[:, H:2 * H])
                    nc.vector.memset(v_sb[:, st, H:H + 1], 1.0)
                    q_n = scratch.tile([128, nh, H], BF16, tag="q_n",
                                       bufs=DEF + 1, name="q_n")
                    norm_scale_copy(q_n, pq, nh)
                    q_ns[st] = q_n

            # ---- Attention + output projection, per 512-query chunk ----
            # (the remaining DEF deferred k/q finishes are spread ahead of the
            # chunk logits that first need them)
            with tc.tile_pool(name="attn", bufs=2) as attn:
                ow_sb = attn.tile([128, nh * 2, D], BF16, tag="ow", bufs=1,
                                  name="ow_sb")
                nc.scalar.dma_start(
                    out=ow_sb, in_=ow_e.rearrange("(i p) d -> p i d", p=128))

                def out_proj(c, qt, encT):
                    for dc in range(D // 512):
                        po = psA.tile([128, 512], F32, tag="pa0", bufs=4,
                                      name="po")
                        for i in range(nh * 2):
                            nc.tensor.matmul(
                                po, encT[:, i, qt * 128:(qt + 1) * 128],
                                ow_sb[:, i, dc * 512:(dc + 1) * 512],
                                start=(i == 0), stop=(i == nh * 2 - 1))
                        o_sb = attn.tile([128, 512], F32, tag="o_sb", bufs=3,
                                         name="o_sb")
                        nc.vector.tensor_copy(o_sb, po)
                        nc.sync.dma_start(
                            out=out_e[(4 * c + qt) * 128:(4 * c + qt + 1) * 128,
                                      dc * 512:(dc + 1) * 512],
                            in_=o_sb)

                def chunk_logits(c):
                    r0 = 4 * c - WT          # first key tile slot (may be <0)
                    R = range(max(0, r0), 4 * c + 4)
                    e_sbs = []
                    for n in range(nh):
                        e_sb = attn.tile([128, WT + 4, 512], BF16, tag="e",
                                         bufs=2 * nh, name="e_sb")
                        e_sbs.append(e_sb)
                        for r in R:
                            t = r - r0
                            lo = max(0, r - 4 * c)
                            hi = min(3, r + WT - 4 * c)
                            qlo, qhi = lo * 128, (hi + 1) * 128
                            plg = psA.tile([128, 512], F32, tag="pa0", bufs=4,
                                           name="plg")
                            nc.tensor.matmul(
                                plg[:, qlo:qhi],
                                kT[:, 0, r * 128:(r + 1) * 128],
                                qT[:, n * 2 + 0, c * 512 + qlo:c * 512 + qhi],
                                start=True, stop=False)
                            nc.tensor.matmul(
                                plg[:, qlo:qhi],
                                kT[:, 1, r * 128:(r + 1) * 128],
                                qT[:, n * 2 + 1, c * 512 + qlo:c * 512 + qhi],
                                start=False, stop=True)
                            nc.scalar.activation(
                                out=e_sb[:, t, qlo:qhi], in_=plg[:, qlo:qhi],
                                func=mybir.ActivationFunctionType.Exp)
                            if 4 * c <= r <= 4 * c + 3:      # diagonal tile
                                dq = r - 4 * c
                                nc.vector.tensor_mul(
                                    e_sb[:, t, dq * 128:(dq + 1) * 128],
                                    e_sb[:, t, dq * 128:(dq + 1) * 128],
                                    tri_sb[:, 0, :])
                            if 4 * c <= r + WT <= 4 * c + 3:  # far (window edge)
                                df = r + WT - 4 * c
                                nc.vector.tensor_mul(
                                    e_sb[:, t, df * 128:(df + 1) * 128],
                                    e_sb[:, t, df * 128:(df + 1) * 128],
                                    tri_sb[:, 1, :])
                    return e_sbs

                def chunk_avout(c, e_sbs):
                    r0 = 4 * c - WT
                    encT = attn.tile([128, nh * 2, 512], BF16, tag="encT",
                                     bufs=2, name="encT")
                    # AV qt-outer, both heads, then the out-projection for
                    # that query tile (one qt late, hiding the enc chain)
                    for qt in range(4):
                        qg = 4 * c + qt
                        rvalid = range(max(0, qg - WT), qg + 1)
                        for n in range(nh):
                            e_sb = e_sbs[n]
                            pe = psA.tile([128, H + 1], F32, tag="pa1", bufs=2,
                                          name="pe")
                            for ri, r in enumerate(rvalid):
                                nc.tensor.matmul(
                                    pe, e_sb[:, r - r0, qt * 128:(qt + 1) * 128],
                                    v_sb[:, r, :],
                                    start=(ri == 0), stop=(ri == len(rvalid) - 1))
                            rden = attn.tile([128, 1], F32, tag="rden", name="rden")
                            nc.vector.reciprocal(rden, pe[:, H:H + 1])
                            enc = attn.tile([128, H], BF16, tag="enc", name="enc")
                            nc.vector.tensor_scalar_mul(enc, pe[:, 0:H], rden)
                            pt = psT.tile([128, 2, 128], BF16, tag="pt", name="pt")
                            for half in range(2):
                                nc.tensor.transpose(
                                    pt[:, half], enc[:, half * HH:(half + 1) * HH],
                                    ident)
                            nc.vector.tensor_copy(
                                encT[:, n * 2:n * 2 + 2, qt * 128:(qt + 1) * 128],
                                pt)
                        if qt > 0:
                            out_proj(c, qt - 1, encT)
                    out_proj(c, 3, encT)

                # chunk-level pipeline: chunk c+1's logits run before chunk
                # c's AV, giving the exp pipeline a full AV+out window to
                # drain before its results are consumed
                fin = list(range(S_TILES - DEF, S_TILES))
                pending = None
                for c in range(NCH):
                    for st in fin[2 * c:2 * c + 2]:
                        k_finish(st)
                        q_finish(st)
                    es = chunk_logits(c)
                    if pending is not None:
                        chunk_avout(*pending)
                    pending = (c, es)
                chunk_avout(*pending)
            scratch_cm.__exit__(None, None, None)
    return nc


# ---------------------------------------------------------------------------
# Host side
# ---------------------------------------------------------------------------

def _rope_tables(pos, scale, extra=1.0):
    """Tables [L, 4, HH] = (C1, S1, C2, S2) with (1+scale) and `extra` folded."""
    frac = 2.0 * np.arange(HH, dtype=np.float64) / H
    ts = ROPE_BASE ** frac
    ang = pos[:, None].astype(np.float64) / ts[None, :]
    sin, cos = np.sin(ang), np.cos(ang)
    g1 = (1.0 + scale[:HH].astype(np.float64)) * extra   # gain on first half
    g2 = (1.0 + scale[HH:].astype(np.float64)) * extra   # gain on second half
    t = np.stack([cos * g1[None, :], sin * g2[None, :],
                  cos * g2[None, :], sin * g1[None, :]], axis=1)
    return t.astype(ml_dtypes.bfloat16)


def _canonical_mask():
    qp = np.arange(T)[:, None]
    kp = np.arange(T)[None, :]
    return (kp <= qp) & ((qp - kp) < WINDOW)


def _numpy_reference(x, q_w, kv_w, q_scale, k_scale, out_w, positions, attn_mask):
    """Slow numpy fallback (only used if attn_mask isn't the canonical
    sliding-window pattern, which never happens for this problem's inputs)."""
    def rms(v, s):
        var = np.mean(np.square(v), axis=-1, keepdims=True)
        return v / np.sqrt(var + EPS) * (1.0 + s)

    def rope_np(v, pos):
        hd = v.shape[-1]
        ts = ROPE_BASE ** (2 * np.arange(hd // 2) / hd)
        ang = pos[..., None] / ts
        ang = ang[..., None, :]
        s, c = np.sin(ang), np.cos(ang)
        f, sec = v[..., :hd // 2], v[..., hd // 2:]
        return np.concatenate([f * c - sec * s, sec * c + f * s], -1)

    q = np.einsum('BTD,NDH->BTNH', x, q_w)
    k = np.einsum('BSD,KDH->BSKH', x, kv_w[0])
    v = np.einsum('BSD,KDH->BSKH', x, kv_w[1])
    q = rope_np(rms(q, q_scale), positions) * H ** -0.5
    k = rope_np(rms(k, k_scale), positions)
    qg = q.reshape(B, T, K, G, H)
    logits = np.einsum('BTKGH,BSKH->BTKGS', qg, k).reshape(B, T, N, T)
    bmask = attn_mask[:, 0][:, :, None, :]
    masked = np.where(bmask, logits, -2.3819763e+38)
    m = masked.max(-1, keepdims=True)
    p = np.exp(masked - m)
    p /= p.sum(-1, keepdims=True)
    enc = np.einsum('BTKGS,BSKH->BTKGH', p.reshape(B, T, K, G, T), v)
    return np.einsum('BTNH,NHD->BTD', enc.reshape(B, T, N, H), out_w)


_NC_CACHE = {}
_IN_MAPS_CACHE = {}


def _get_nc():
    if "nc" not in _NC_CACHE:
        nc = build_nc()
        nc.finalize()
        _NC_CACHE["nc"] = nc
    return _NC_CACHE["nc"]


def kernel(x, q_w, kv_w, q_scale, k_scale, out_w, positions, attn_mask):
    bf16 = ml_dtypes.bfloat16

    if not np.array_equal(
            np.asarray(attn_mask),
            np.broadcast_to(_canonical_mask()[None, None], (B, 1, T, T))):
        return _numpy_reference(
            np.asarray(x, np.float32), np.asarray(q_w, np.float32),
            np.asarray(kv_w, np.float32), np.asarray(q_scale, np.float32),
            np.asarray(k_scale, np.float32), np.asarray(out_w, np.float32),
            np.asarray(positions), np.asarray(attn_mask)).astype(np.float32)

    tri = np.zeros((2, 128, 128), np.float32)
    tri[0] = np.triu(np.ones((128, 128)))        # diag tile: key <= query
    tri[1] = np.tril(np.ones((128, 128)), -1)    # far tile:  key > query
    tri = tri.astype(bf16)

    in_maps = []
    for c in range(8):
        b, j = divmod(c, 4)
        xT = np.ascontiguousarray(np.asarray(x[b]).T).astype(bf16)
        kvw = np.ascontiguousarray(
            np.concatenate([kv_w[0, j], kv_w[1, j]], axis=1)).astype(bf16)
        qw = np.ascontiguousarray(
            q_w[2 * j:2 * j + 2].transpose(1, 0, 2).reshape(D, 2 * H)).astype(bf16)
        ow = np.ascontiguousarray(
            out_w[2 * j:2 * j + 2].reshape(2 * H, D)).astype(bf16)
        pos = np.asarray(positions[b])
        tq = _rope_tables(pos, np.asarray(q_scale), extra=H ** -0.5)
        tk = _rope_tables(pos, np.asarray(k_scale))
        in_maps.append({"xT": xT, "kvw": kvw, "qw": qw, "ow": ow,
                        "tq": tq, "tk": tk, "tri": tri})

    from concourse.bass_utils import run_bass_kernel_spmd
    _IN_MAPS_CACHE["in_maps"] = in_maps
    nc = _get_nc()
    res = run_bass_kernel_spmd(nc, in_maps, list(range(8)))
    out = np.empty((B, T, D), np.float32)
    for b in range(B):
        acc = res.results[4 * b]["out"].astype(np.float32)
        for j in range(1, 4):
            acc = acc + res.results[4 * b + j]["out"]
        out[b] = acc
    return out
